# revision 4
# baseline (speedup 1.0000x reference)
"""BERT forward (2 layers, B=8, S=1024, D=768, H=12, FF=3072) on 8 trn2 cores.

Data-parallel over batch: core c computes batch row c end-to-end (no
collectives). Outputs: pooled [8,768] f32 and attention probs
[2,8,12,1024,1024] f32 (computed/stored bf16, upcast on host).

Per-core layouts:
  h_tok  [128, 8, 768]  f32  token-major residual stream (token = t*128+p)
  hT     [128, 6, 1024] bf16 feature-major post-LN hidden (feature = j*128+p)
  QT/KT  [128, 6, 1024] bf16 feature-major (head h rows h*64..h*64+63)
  V      [128, 8, 768]  bf16 token-major
  ctxT   [128, 6, 1024] bf16 feature-major attention context
Matmul forms: F1 (lhsT=W slice, rhs=hT) -> feature-major out;
F2 (lhsT=hT/ctxT slice, rhs=W) -> token-major out.
Softmax: scores both orientations; exp on ACT with accum_out giving Z;
1/Z broadcast along the free dim via a DRAM-transposed roundtrip.
"""
import sys

sys.path.insert(0, '/opt/trn_rl_repo')

import numpy as np
import ml_dtypes

import concourse.bass as bass
import concourse.tile as tile
from concourse import mybir, bacc
from concourse.bass_utils import run_bass_kernel_spmd
from concourse.masks import make_identity

BF16 = mybir.dt.bfloat16
F32 = mybir.dt.float32
I32 = mybir.dt.int32
Exp = mybir.ActivationFunctionType.Exp
Ln = mybir.ActivationFunctionType.Ln
Gelu = mybir.ActivationFunctionType.Gelu
Tanh = mybir.ActivationFunctionType.Tanh
ADD = mybir.AluOpType.add
SUB = mybir.AluOpType.subtract
MUL = mybir.AluOpType.mult

B, S, D, H, L, FF, V, DK = 8, 1024, 768, 12, 2, 3072, 30522, 64
NT = S // 128   # 8 token tiles
ND = D // 128   # 6 feature tiles
NC = FF // 128  # 24 ffn chunks
EPS_EMB = 1e-12
EPS_LN = 1e-5
N_CORES = 8
NBF = np.dtype(ml_dtypes.bfloat16)


def build(flags):
    """Trace + compile the per-core kernel. flags: (use_mask, use_bias, use_lnw)."""
    use_mask, use_bias, use_lnw = flags
    nc = bacc.Bacc("TRN2", target_bir_lowering=False, debug=False,
                   num_devices=N_CORES)

    # ---- DRAM I/O ----
    ids_d = nc.dram_tensor("ids", [S], I32, kind="ExternalInput")
    wemb_d = nc.dram_tensor("wemb", [V, D], F32, kind="ExternalInput")
    pos_d = nc.dram_tensor("pos", [S, D], F32, kind="ExternalInput")
    type0_d = nc.dram_tensor("type0", [1, D], F32, kind="ExternalInput")
    eg_d = nc.dram_tensor("eg", [1, D], F32, kind="ExternalInput")
    eb_d = nc.dram_tensor("eb", [1, D], F32, kind="ExternalInput")
    wq_d = nc.dram_tensor("wq", [L, D, D], BF16, kind="ExternalInput")
    wk_d = nc.dram_tensor("wk", [L, D, D], BF16, kind="ExternalInput")
    wv_d = nc.dram_tensor("wv", [L, D, D], BF16, kind="ExternalInput")
    wo_d = nc.dram_tensor("wo", [L, D, D], BF16, kind="ExternalInput")
    w1_d = nc.dram_tensor("w1", [L, D, FF], BF16, kind="ExternalInput")
    w2_d = nc.dram_tensor("w2", [L, FF, D], BF16, kind="ExternalInput")
    wp_d = nc.dram_tensor("wp", [D, D], BF16, kind="ExternalInput")
    bp_d = nc.dram_tensor("bp", [D], F32, kind="ExternalInput")
    if use_bias:
        bq_d = nc.dram_tensor("bq", [L, D], F32, kind="ExternalInput")
        bk_d = nc.dram_tensor("bk", [L, D], F32, kind="ExternalInput")
        bv_d = nc.dram_tensor("bv", [L, D], BF16, kind="ExternalInput")
        bo_d = nc.dram_tensor("bo", [L, D], BF16, kind="ExternalInput")
        b1_d = nc.dram_tensor("b1", [L, FF], F32, kind="ExternalInput")
        b2_d = nc.dram_tensor("b2", [L, D], BF16, kind="ExternalInput")
    if use_lnw:
        l1g_d = nc.dram_tensor("l1g", [L, 1, D], F32, kind="ExternalInput")
        l1b_d = nc.dram_tensor("l1b", [L, 1, D], F32, kind="ExternalInput")
        l2g_d = nc.dram_tensor("l2g", [L, 1, D], F32, kind="ExternalInput")
        l2b_d = nc.dram_tensor("l2b", [L, 1, D], F32, kind="ExternalInput")
    if use_mask:
        mb_d = nc.dram_tensor("mb", [1, S], F32, kind="ExternalInput")

    attn_d = nc.dram_tensor("attn", [L, H, S, S], BF16, kind="ExternalOutput")
    pool_d = nc.dram_tensor("pool", [D], F32, kind="ExternalOutput")

    with tile.TileContext(nc) as tc:
        import contextlib
        ctx = contextlib.ExitStack()
        with ctx:
            sing = ctx.enter_context(tc.tile_pool(name="sing", bufs=1))
            hpool = ctx.enter_context(tc.tile_pool(name="h", bufs=1))
            htp = ctx.enter_context(tc.tile_pool(name="hT", bufs=1))
            qkvp = ctx.enter_context(tc.tile_pool(name="qkv", bufs=1))
            wpool = ctx.enter_context(tc.tile_pool(name="w", bufs=4))
            wfp = ctx.enter_context(tc.tile_pool(name="wf", bufs=4))
            bpool = ctx.enter_context(tc.tile_pool(name="b", bufs=8))
            embp = ctx.enter_context(tc.tile_pool(name="emb", bufs=2))
            expp = ctx.enter_context(tc.tile_pool(name="exp", bufs=3))
            expt = ctx.enter_context(tc.tile_pool(name="expT", bufs=4))
            attp = ctx.enter_context(tc.tile_pool(name="attn", bufs=3))
            izp = ctx.enter_context(tc.tile_pool(name="iz", bufs=2))
            ubfp = ctx.enter_context(tc.tile_pool(name="ubf", bufs=3))
            misc = ctx.enter_context(tc.tile_pool(name="misc", bufs=4))
            lnp = ctx.enter_context(tc.tile_pool(name="ln", bufs=2))
            ps2 = ctx.enter_context(tc.tile_pool(name="ps2", bufs=4, space="PSUM"))
            drp = ctx.enter_context(tc.tile_pool(name="dr", bufs=2, space="DRAM"))

            ident = sing.tile([128, 128], BF16)
            make_identity(nc, ident)
            if use_bias:
                ones1 = sing.tile([1, 128], BF16)
                nc.vector.memset(ones1, 1.0)
            eps_emb = sing.tile([128, 1], F32)
            nc.vector.memset(eps_emb, EPS_EMB)
            eps_ln = sing.tile([128, 1], F32)
            nc.vector.memset(eps_ln, EPS_LN)

            def bcast_row(dram_row, n, dt=F32, pool=None):
                # [1, n] DRAM row -> [128, n] SBUF, replicated on partitions
                t = (pool or sing).tile([128, n], dt)
                src = bass.AP(tensor=dram_row.tensor,
                              offset=dram_row.offset, ap=[[0, 128], [1, n]])
                nc.gpsimd.dma_start(out=t, in_=src)
                return t

            type_bc = bcast_row(type0_d[0], D)
            if use_lnw:
                eg_bc = bcast_row(eg_d[0], D)
                eb_bc = bcast_row(eb_d[0], D)
            if use_mask:
                mb_bc = bcast_row(mb_d[0], S)
                mb_sb = misc.tile([128, NT], F32)
                nc.sync.dma_start(out=mb_sb,
                                  in_=mb_d[0].rearrange("(t p) -> p t", p=128))

            h_tok = hpool.tile([128, NT, D], F32)
            hT = htp.tile([128, ND, S], BF16, tag="hT")

            # ---------- layer norm helper (token-major, batched sqrt) ----------
            def layer_norm_tiles(get_x, eps_tile, g_bc, b_bc, dst_hT):
                """In-place LN on get_x(t) [128,768] f32; also writes the
                feature-major bf16 transpose into dst_hT [128,6,1024]."""
                mv_all = lnp.tile([128, NT, 2], F32, tag="mv")
                stats = lnp.tile([128, 3, 6], F32, tag="st")
                for t in range(NT):
                    xg = get_x(t).rearrange("p (n f) -> p n f", f=256)
                    for i in range(3):
                        nc.vector.bn_stats(out=stats[:, i, :], in_=xg[:, i, :])
                    nc.vector.bn_aggr(out=mv_all[:, t, :], in_=stats)
                lnv = lnp.tile([128, NT], F32, tag="lnv")
                nc.scalar.activation(out=lnv, in_=mv_all[:, :, 1], func=Ln,
                                     bias=eps_tile)
                rstd = lnp.tile([128, NT], F32, tag="rstd")
                nc.scalar.activation(out=rstd, in_=lnv, func=Exp, scale=-0.5)
                for t in range(NT):
                    o = get_x(t)
                    nc.vector.tensor_scalar(out=o, in0=o,
                                            scalar1=mv_all[:, t, 0:1],
                                            scalar2=rstd[:, t:t + 1],
                                            op0=SUB, op1=MUL)
                    if g_bc is not None:
                        nc.vector.tensor_tensor(out=o, in0=o, in1=g_bc, op=MUL)
                        nc.vector.tensor_tensor(out=o, in0=o, in1=b_bc, op=ADD)
                    bt = lnp.tile([128, D], BF16, tag="hbf")
                    nc.vector.tensor_copy(out=bt, in_=o)
                    tp = ps2.tile([128, 2048], BF16, tag="ps2")
                    for j in range(ND):
                        nc.tensor.transpose(out=tp[:, j * 128:(j + 1) * 128],
                                            in_=bt[:, j * 128:(j + 1) * 128],
                                            identity=ident)
                    nc.vector.tensor_copy(
                        out=dst_hT[:, :, t * 128:(t + 1) * 128],
                        in_=tp[:, 0:768].rearrange("p (j n) -> p j n", n=128))

            # ---------- embeddings ----------
            ids_sb = sing.tile([128, NT], I32)
            nc.sync.dma_start(out=ids_sb, in_=ids_d.rearrange("(t p) -> p t", p=128))
            for t in range(NT):
                g_t = embp.tile([128, D], F32, tag="gath")
                nc.gpsimd.indirect_dma_start(
                    out=g_t[:], out_offset=None, in_=wemb_d[:],
                    in_offset=bass.IndirectOffsetOnAxis(ap=ids_sb[:, t:t + 1], axis=0))
                p_t = embp.tile([128, D], F32, tag="pos")
                nc.sync.dma_start(out=p_t, in_=pos_d[t * 128:(t + 1) * 128, :])
                x_t = h_tok[:, t, :]
                nc.vector.tensor_tensor(out=x_t, in0=g_t, in1=p_t, op=ADD)
                nc.vector.tensor_tensor(out=x_t, in0=x_t, in1=type_bc, op=ADD)
            layer_norm_tiles(lambda t: h_tok[:, t, :], eps_emb,
                             eg_bc if use_lnw else None,
                             eb_bc if use_lnw else None, hT)

            # ---------- layers ----------
            for l in range(L):
                wq_sb = wpool.tile([128, ND, D], BF16, tag="w")
                nc.sync.dma_start(out=wq_sb,
                                  in_=wq_d[l].rearrange("(k p) n -> p k n", p=128))
                wk_sb = wpool.tile([128, ND, D], BF16, tag="w")
                nc.sync.dma_start(out=wk_sb,
                                  in_=wk_d[l].rearrange("(k p) n -> p k n", p=128))
                wv_sb = wpool.tile([128, ND, D], BF16, tag="w")
                nc.sync.dma_start(out=wv_sb,
                                  in_=wv_d[l].rearrange("(k p) n -> p k n", p=128))
                if use_bias:
                    bq_sb = bpool.tile([128, ND], F32, tag="bq")
                    nc.sync.dma_start(out=bq_sb,
                                      in_=bq_d[l].rearrange("(k p) -> p k", p=128))
                    bk_sb = bpool.tile([128, ND], F32, tag="bk")
                    nc.sync.dma_start(out=bk_sb,
                                      in_=bk_d[l].rearrange("(k p) -> p k", p=128))
                    bv_row = bpool.tile([1, D], BF16, tag="bv")
                    nc.sync.dma_start(out=bv_row, in_=bv_d[l:l + 1, :])
                    bo_row = bpool.tile([1, D], BF16, tag="bo")
                    nc.sync.dma_start(out=bo_row, in_=bo_d[l:l + 1, :])
                    b1_sb = bpool.tile([128, NC], F32, tag="b1")
                    nc.sync.dma_start(out=b1_sb,
                                      in_=b1_d[l].rearrange("(c p) -> p c", p=128))
                    b2_row = bpool.tile([1, D], BF16, tag="b2")
                    nc.sync.dma_start(out=b2_row, in_=b2_d[l:l + 1, :])
                if use_lnw:
                    l1g_bc = bcast_row(l1g_d[l, 0], D, pool=bpool)
                    l1b_bc = bcast_row(l1b_d[l, 0], D, pool=bpool)
                    l2g_bc = bcast_row(l2g_d[l, 0], D, pool=bpool)
                    l2b_bc = bcast_row(l2b_d[l, 0], D, pool=bpool)

                # --- QKV projections ---
                QT = qkvp.tile([128, ND, S], BF16, tag="QT")
                KT = qkvp.tile([128, ND, S], BF16, tag="KT")
                V_sb = qkvp.tile([128, NT, D], BF16, tag="V")
                for (W, bias_sb, out_t) in ((wq_sb, None, QT), (wk_sb, None, KT)):
                    bsb = None
                    if use_bias:
                        bsb = bq_sb if out_t is QT else bk_sb
                    for j in range(ND):
                        ps = ps2.tile([128, 1024], F32, tag="ps2")
                        for half in range(2):
                            for k in range(ND):
                                nc.tensor.matmul(
                                    out=ps[:, half * 512:(half + 1) * 512],
                                    lhsT=W[:, k, j * 128:(j + 1) * 128],
                                    rhs=hT[:, k, half * 512:(half + 1) * 512],
                                    start=(k == 0), stop=(k == ND - 1))
                        if bsb is not None:
                            nc.vector.tensor_scalar(out=out_t[:, j, :], in0=ps,
                                                    scalar1=bsb[:, j:j + 1],
                                                    scalar2=None, op0=ADD)
                        else:
                            nc.vector.tensor_copy(out=out_t[:, j, :], in_=ps)
                for t in range(NT):
                    ps = ps2.tile([128, 1024], F32, tag="ps2")
                    for nh in range(2):
                        o0 = nh * 512
                        for k in range(ND):
                            nc.tensor.matmul(
                                out=ps[:, o0:o0 + 384],
                                lhsT=hT[:, k, t * 128:(t + 1) * 128],
                                rhs=wv_sb[:, k, nh * 384:(nh + 1) * 384],
                                start=(k == 0),
                                stop=(k == ND - 1 and not use_bias))
                        if use_bias:
                            nc.tensor.matmul(
                                out=ps[:, o0:o0 + 384], lhsT=ones1,
                                rhs=bv_row[0:1, nh * 384:(nh + 1) * 384],
                                start=False, stop=True)
                    nc.vector.tensor_copy(out=V_sb[:, t, 0:384], in_=ps[:, 0:384])
                    nc.vector.tensor_copy(out=V_sb[:, t, 384:768], in_=ps[:, 512:896])

                wo_sb = wpool.tile([128, ND, D], BF16, tag="w")
                nc.sync.dma_start(out=wo_sb,
                                  in_=wo_d[l].rearrange("(k p) n -> p k n", p=128))

                # --- attention phase A: scores [q,k], exp, attn out ---
                zinv = misc.tile([128, H * NT], F32, tag="zinv")
                for hp in range(H // 2):
                    A, Bh = 2 * hp, 2 * hp + 1
                    for t in range(NT):
                        for (hd, prow, tp_) in ((A, 0, (0, 0)), (Bh, 64, (64, 0))):
                            sps = ps2.tile([128, 1024], F32, tag="ps2")
                            for half in range(2):
                                nc.tensor.matmul(
                                    out=sps[:, half * 512:(half + 1) * 512],
                                    lhsT=QT[prow:prow + 64, hp, t * 128:(t + 1) * 128],
                                    rhs=KT[prow:prow + 64, hp, half * 512:(half + 1) * 512],
                                    start=True, stop=True, tile_position=tp_)
                            if use_mask:
                                nc.vector.tensor_tensor(out=sps, in0=sps,
                                                        in1=mb_bc, op=ADD)
                            idx = hd * NT + t
                            ex = expp.tile([128, 1024], F32, tag="exp")
                            nc.scalar.activation(out=ex, in_=sps, func=Exp,
                                                 scale=0.125,
                                                 accum_out=zinv[:, idx:idx + 1])
                            nc.vector.reciprocal(out=zinv[:, idx:idx + 1],
                                                 in_=zinv[:, idx:idx + 1])
                            at = attp.tile([128, 1024], BF16, tag="attn")
                            nc.gpsimd.tensor_scalar_mul(out=at, in0=ex,
                                                        scalar1=zinv[:, idx:idx + 1])
                            nc.scalar.dma_start(
                                out=attn_d[l, hd, t * 128:(t + 1) * 128, :], in_=at)
                # 1/Z -> DRAM transposed: zd[c, r] = zinv[r, c]
                zd = drp.tile([H * NT, 128], F32, tag="zd")
                nc.sync.dma_start(
                    out=bass.AP(tensor=zd.tensor, offset=zd.offset,
                                ap=[[1, 128], [128, H * NT]]),
                    in_=zinv)

                # --- attention phase B/C: scores [k,q], exp, ctx ---
                ctxT = qkvp.tile([128, ND, S], BF16, tag="ctxT")
                for hp in range(H // 2):
                    A, Bh = 2 * hp, 2 * hp + 1
                    izb = izp.tile([128, 2, 1024], BF16, tag="iz")
                    nc.gpsimd.dma_start(
                        out=izb.rearrange("p a q -> p (a q)"),
                        in_=bass.AP(tensor=zd.tensor,
                                    offset=zd.offset + A * NT * 128,
                                    ap=[[0, 128], [1, 2048]]))
                    cps = ps2.tile([128, 1024], F32, tag="ps2")
                    for kt in range(NT):
                        for (hd, prow, tp_, crow) in (
                                (A, 0, (0, 0), 0), (Bh, 64, (64, 0), 64)):
                            sps = ps2.tile([128, 1024], F32, tag="ps2")
                            for half in range(2):
                                nc.tensor.matmul(
                                    out=sps[:, half * 512:(half + 1) * 512],
                                    lhsT=KT[prow:prow + 64, hp, kt * 128:(kt + 1) * 128],
                                    rhs=QT[prow:prow + 64, hp, half * 512:(half + 1) * 512],
                                    start=True, stop=True, tile_position=tp_)
                            et = expt.tile([128, 1024], BF16, tag="expT")
                            if use_mask:
                                nc.scalar.activation(out=et, in_=sps, func=Exp,
                                                     scale=0.125,
                                                     bias=mb_sb[:, kt:kt + 1])
                            else:
                                nc.scalar.activation(out=et, in_=sps, func=Exp,
                                                     scale=0.125)
                            for half in range(2):
                                nc.tensor.matmul(
                                    out=cps[crow:crow + 64,
                                            half * 512:(half + 1) * 512],
                                    lhsT=V_sb[:, kt, hd * 64:(hd + 1) * 64],
                                    rhs=et[:, half * 512:(half + 1) * 512],
                                    start=(kt == 0), stop=(kt == NT - 1),
                                    tile_position=(0, crow))
                    nc.vector.tensor_tensor(out=ctxT[0:64, hp, :], in0=cps[0:64, :],
                                            in1=izb[0:64, 0, :], op=MUL)
                    nc.vector.tensor_tensor(out=ctxT[64:128, hp, :],
                                            in0=cps[64:128, :],
                                            in1=izb[64:128, 1, :], op=MUL)

                # --- Wo + residual ---
                for t in range(NT):
                    ps = ps2.tile([128, 1024], F32, tag="ps2")
                    for nh in range(2):
                        o0 = nh * 512
                        for k in range(ND):
                            nc.tensor.matmul(
                                out=ps[:, o0:o0 + 384],
                                lhsT=ctxT[:, k, t * 128:(t + 1) * 128],
                                rhs=wo_sb[:, k, nh * 384:(nh + 1) * 384],
                                start=(k == 0),
                                stop=(k == ND - 1 and not use_bias))
                        if use_bias:
                            nc.tensor.matmul(
                                out=ps[:, o0:o0 + 384], lhsT=ones1,
                                rhs=bo_row[0:1, nh * 384:(nh + 1) * 384],
                                start=False, stop=True)
                        nc.vector.tensor_tensor(
                            out=h_tok[:, t, nh * 384:(nh + 1) * 384],
                            in0=ps[:, o0:o0 + 384],
                            in1=h_tok[:, t, nh * 384:(nh + 1) * 384], op=ADD)
                h1T = htp.tile([128, ND, S], BF16, tag="hT")
                layer_norm_tiles(lambda t: h_tok[:, t, :], eps_ln,
                                 l1g_bc if use_lnw else None,
                                 l1b_bc if use_lnw else None, h1T)

                # --- FFN (token quarters of 256) ---
                for q4 in range(4):
                    o2 = [ps2.tile([128, 1024], F32, tag="ps2", name=f"o2_{q4}_{i}")
                          for i in range(2)]
                    for c in range(NC):
                        w1c = wfp.tile([128, ND, 128], BF16, tag="w1c")
                        nc.sync.dma_start(
                            out=w1c,
                            in_=w1_d[l][:, c * 128:(c + 1) * 128]
                            .rearrange("(k p) n -> p k n", p=128))
                        w2c = wfp.tile([128, D], BF16, tag="w2c")
                        nc.sync.dma_start(out=w2c, in_=w2_d[l][c * 128:(c + 1) * 128, :])
                        ups = ps2.tile([128, 1024], F32, tag="ps2")
                        for k in range(ND):
                            nc.tensor.matmul(
                                out=ups[:, 0:256],
                                lhsT=w1c[:, k, :],
                                rhs=h1T[:, k, q4 * 256:(q4 + 1) * 256],
                                start=(k == 0), stop=(k == ND - 1))
                        ub = ubfp.tile([128, 256], BF16, tag="ubf")
                        if use_bias:
                            nc.scalar.activation(out=ub, in_=ups[:, 0:256],
                                                 func=Gelu,
                                                 bias=b1_sb[:, c:c + 1])
                        else:
                            nc.scalar.activation(out=ub, in_=ups[:, 0:256],
                                                 func=Gelu)
                        for ts_ in range(2):
                            for nh in range(2):
                                nc.tensor.matmul(
                                    out=o2[ts_][:, nh * 512:nh * 512 + 384],
                                    lhsT=ub[:, ts_ * 128:(ts_ + 1) * 128],
                                    rhs=w2c[:, nh * 384:(nh + 1) * 384],
                                    start=(c == 0),
                                    stop=(c == NC - 1 and not use_bias))
                    for ts_ in range(2):
                        t = q4 * 2 + ts_
                        for nh in range(2):
                            if use_bias:
                                nc.tensor.matmul(
                                    out=o2[ts_][:, nh * 512:nh * 512 + 384],
                                    lhsT=ones1,
                                    rhs=b2_row[0:1, nh * 384:(nh + 1) * 384],
                                    start=False, stop=True)
                            nc.vector.tensor_tensor(
                                out=h_tok[:, t, nh * 384:(nh + 1) * 384],
                                in0=o2[ts_][:, nh * 512:nh * 512 + 384],
                                in1=h_tok[:, t, nh * 384:(nh + 1) * 384], op=ADD)
                hT = htp.tile([128, ND, S], BF16, tag="hT")
                layer_norm_tiles(lambda t: h_tok[:, t, :], eps_ln,
                                 l2g_bc if use_lnw else None,
                                 l2b_bc if use_lnw else None, hT)

            # ---------- pooler ----------
            wp_sb = wpool.tile([128, ND, D], BF16, tag="w")
            nc.sync.dma_start(out=wp_sb,
                              in_=wp_d.rearrange("(k p) n -> p k n", p=128))
            bp_sb = misc.tile([128, ND], F32, tag="bp")
            nc.sync.dma_start(out=bp_sb, in_=bp_d.rearrange("(j p) -> p j", p=128))
            pool_sb = misc.tile([128, ND], F32, tag="pool")
            pps = ps2.tile([128, 1024], F32, tag="ps2")
            for j in range(ND):
                for k in range(ND):
                    nc.tensor.matmul(out=pps[:, j:j + 1],
                                     lhsT=wp_sb[:, k, j * 128:(j + 1) * 128],
                                     rhs=hT[:, k, 0:1],
                                     start=(k == 0), stop=(k == ND - 1))
                nc.scalar.activation(out=pool_sb[:, j:j + 1], in_=pps[:, j:j + 1],
                                     func=Tanh, bias=bp_sb[:, j:j + 1])
            nc.sync.dma_start(out=pool_d.rearrange("(j p) -> p j", p=128),
                              in_=pool_sb)

    nc.compile()
    return nc


_CACHE = {}


def _get_kernel(flags):
    if flags not in _CACHE:
        _CACHE[flags] = build(flags)
    return _CACHE[flags]


def kernel(input_ids, attention_mask, word_emb, pos_emb, type_emb,
           emb_ln_g, emb_ln_b, Wq, bq, Wk, bk, Wv, bv, Wo, bo,
           W1, b1, W2, b2, ln1_g, ln1_b, ln2_g, ln2_b, Wp, bp,
           _trace=False, _tmpdir=None):
    input_ids = np.asarray(input_ids)
    attention_mask = np.asarray(attention_mask)
    f32 = lambda x: np.ascontiguousarray(np.asarray(x), dtype=np.float32)
    bf = lambda x: np.ascontiguousarray(np.asarray(x)).astype(NBF)

    use_mask = not bool((attention_mask == 1).all())
    use_bias = any(bool(np.asarray(b).any()) for b in (bq, bk, bv, bo, b1, b2))
    use_lnw = not (bool((np.asarray(emb_ln_g) == 1).all())
                   and not np.asarray(emb_ln_b).any()
                   and bool((np.asarray(ln1_g) == 1).all())
                   and not np.asarray(ln1_b).any()
                   and bool((np.asarray(ln2_g) == 1).all())
                   and not np.asarray(ln2_b).any())
    flags = (use_mask, use_bias, use_lnw)
    nc = _get_kernel(flags)

    shared = {
        "wemb": f32(word_emb), "pos": f32(pos_emb)[:S],
        "type0": f32(type_emb)[0:1],
        "eg": f32(emb_ln_g).reshape(1, D), "eb": f32(emb_ln_b).reshape(1, D),
        "wq": bf(Wq), "wk": bf(Wk), "wv": bf(Wv), "wo": bf(Wo),
        "w1": bf(W1), "w2": bf(W2), "wp": bf(Wp), "bp": f32(bp),
    }
    if use_bias:
        shared.update({"bq": f32(bq), "bk": f32(bk), "bv": bf(bv),
                       "bo": bf(bo), "b1": f32(b1), "b2": bf(b2)})
    if use_lnw:
        shared.update({"l1g": f32(ln1_g).reshape(L, 1, D),
                       "l1b": f32(ln1_b).reshape(L, 1, D),
                       "l2g": f32(ln2_g).reshape(L, 1, D),
                       "l2b": f32(ln2_b).reshape(L, 1, D)})

    in_maps = []
    for c in range(N_CORES):
        m = dict(shared)
        m["ids"] = np.ascontiguousarray(input_ids[c], dtype=np.int32)
        if use_mask:
            m["mb"] = ((1.0 - attention_mask[c].astype(np.float32))
                       * -1e9).reshape(1, S)
        in_maps.append(m)

    kw = {}
    if _trace:
        kw = {"trace": True, "tmpdir": _tmpdir}
    res = run_bass_kernel_spmd(nc, in_maps, core_ids=list(range(N_CORES)), **kw)

    pooled = np.stack([res.results[c]["pool"] for c in range(N_CORES)])
    attn = np.stack([res.results[c]["attn"].astype(np.float32)
                     for c in range(N_CORES)], axis=1)
    kernel.last_exec_time_ns = res.exec_time_ns
    return pooled, attn


# revision 5
# speedup vs baseline: 3.0788x; 3.0788x over previous
"""BERT forward (2 layers, B=8, S=1024, D=768, H=12, FF=3072) on 8 trn2 cores.

Data-parallel over batch: core c computes batch row c end-to-end (no
collectives). Outputs: pooled [8,768] f32 and attention probs
[2,8,12,1024,1024] f32 (computed/stored bf16, upcast on host).

Per-core layouts:
  h_tok  [128, 8, 768]  f32  token-major residual stream (token = t*128+p)
  hT     [128, 6, 1024] bf16 feature-major post-LN hidden (feature = j*128+p)
  QT/KT  [128, 6, 1024] bf16 feature-major (head h rows h*64..h*64+63)
  V      [128, 8, 768]  bf16 token-major
  ctxT   [128, 6, 1024] bf16 feature-major attention context
Matmul forms: F1 (lhsT=W slice, rhs=hT) -> feature-major out;
F2 (lhsT=hT/ctxT slice, rhs=W) -> token-major out.
Softmax: scores both orientations; exp on ACT with accum_out giving Z;
1/Z broadcast along the free dim via a DRAM-transposed roundtrip.
"""
import sys

sys.path.insert(0, '/opt/trn_rl_repo')

import numpy as np
import ml_dtypes

import concourse.bass as bass
import concourse.tile as tile
from concourse import mybir, bacc
from concourse.bass_utils import run_bass_kernel_spmd
from concourse.masks import make_identity

BF16 = mybir.dt.bfloat16
F32 = mybir.dt.float32
I32 = mybir.dt.int32
Exp = mybir.ActivationFunctionType.Exp
Ln = mybir.ActivationFunctionType.Ln
Gelu = mybir.ActivationFunctionType.Gelu
Tanh = mybir.ActivationFunctionType.Tanh
ADD = mybir.AluOpType.add
SUB = mybir.AluOpType.subtract
MUL = mybir.AluOpType.mult

B, S, D, H, L, FF, V, DK = 8, 1024, 768, 12, 2, 3072, 30522, 64
NT = S // 128   # 8 token tiles
ND = D // 128   # 6 feature tiles
NC = FF // 128  # 24 ffn chunks
EPS_EMB = 1e-12
EPS_LN = 1e-5
N_CORES = 8
NBF = np.dtype(ml_dtypes.bfloat16)


def build(flags):
    """Trace + compile the per-core kernel. flags: (use_mask, use_bias, use_lnw)."""
    use_mask, use_bias, use_lnw = flags
    nc = bacc.Bacc("TRN2", target_bir_lowering=False, debug=False,
                   num_devices=N_CORES)

    # ---- DRAM I/O ----
    ids_d = nc.dram_tensor("ids", [S], I32, kind="ExternalInput")
    wemb_d = nc.dram_tensor("wemb", [V, D], F32, kind="ExternalInput")
    pos_d = nc.dram_tensor("pos", [S, D], F32, kind="ExternalInput")
    type0_d = nc.dram_tensor("type0", [1, D], F32, kind="ExternalInput")
    eg_d = nc.dram_tensor("eg", [1, D], F32, kind="ExternalInput")
    eb_d = nc.dram_tensor("eb", [1, D], F32, kind="ExternalInput")
    wq_d = nc.dram_tensor("wq", [L, D, D], BF16, kind="ExternalInput")
    wk_d = nc.dram_tensor("wk", [L, D, D], BF16, kind="ExternalInput")
    wv_d = nc.dram_tensor("wv", [L, D, D], BF16, kind="ExternalInput")
    wo_d = nc.dram_tensor("wo", [L, D, D], BF16, kind="ExternalInput")
    w1_d = nc.dram_tensor("w1", [L, D, FF], BF16, kind="ExternalInput")
    w2_d = nc.dram_tensor("w2", [L, FF, D], BF16, kind="ExternalInput")
    wp_d = nc.dram_tensor("wp", [D, D], BF16, kind="ExternalInput")
    bp_d = nc.dram_tensor("bp", [D], F32, kind="ExternalInput")
    if use_bias:
        bq_d = nc.dram_tensor("bq", [L, D], F32, kind="ExternalInput")
        bk_d = nc.dram_tensor("bk", [L, D], F32, kind="ExternalInput")
        bv_d = nc.dram_tensor("bv", [L, D], BF16, kind="ExternalInput")
        bo_d = nc.dram_tensor("bo", [L, D], BF16, kind="ExternalInput")
        b1_d = nc.dram_tensor("b1", [L, FF], F32, kind="ExternalInput")
        b2_d = nc.dram_tensor("b2", [L, D], BF16, kind="ExternalInput")
    if use_lnw:
        l1g_d = nc.dram_tensor("l1g", [L, 1, D], F32, kind="ExternalInput")
        l1b_d = nc.dram_tensor("l1b", [L, 1, D], F32, kind="ExternalInput")
        l2g_d = nc.dram_tensor("l2g", [L, 1, D], F32, kind="ExternalInput")
        l2b_d = nc.dram_tensor("l2b", [L, 1, D], F32, kind="ExternalInput")
    if use_mask:
        mb_d = nc.dram_tensor("mb", [1, S], F32, kind="ExternalInput")

    attn_d = nc.dram_tensor("attn", [L, H, S, S], BF16, kind="ExternalOutput")
    pool_d = nc.dram_tensor("pool", [D], F32, kind="ExternalOutput")

    with tile.TileContext(nc) as tc:
        import contextlib
        ctx = contextlib.ExitStack()
        with ctx:
            sing = ctx.enter_context(tc.tile_pool(name="sing", bufs=1))
            hpool = ctx.enter_context(tc.tile_pool(name="h", bufs=1))
            htp = ctx.enter_context(tc.tile_pool(name="hT", bufs=1))
            qkvp = ctx.enter_context(tc.tile_pool(name="qkv", bufs=1))
            wpool = ctx.enter_context(tc.tile_pool(name="w", bufs=4))
            wfp = ctx.enter_context(tc.tile_pool(name="wf", bufs=4))
            bpool = ctx.enter_context(tc.tile_pool(name="b", bufs=8))
            embp = ctx.enter_context(tc.tile_pool(name="emb", bufs=2))
            expp = ctx.enter_context(tc.tile_pool(name="exp", bufs=3))
            expt = ctx.enter_context(tc.tile_pool(name="expT", bufs=4))
            attp = ctx.enter_context(tc.tile_pool(name="attn", bufs=3))
            izp = ctx.enter_context(tc.tile_pool(name="iz", bufs=2))
            ubfp = ctx.enter_context(tc.tile_pool(name="ubf", bufs=3))
            misc = ctx.enter_context(tc.tile_pool(name="misc", bufs=4))
            lnp = ctx.enter_context(tc.tile_pool(name="ln", bufs=2))
            ps2 = ctx.enter_context(tc.tile_pool(name="ps2", bufs=4, space="PSUM"))
            drp = ctx.enter_context(tc.tile_pool(name="dr", bufs=2, space="DRAM"))

            ident = sing.tile([128, 128], BF16)
            make_identity(nc, ident)
            if use_bias:
                ones1 = sing.tile([1, 128], BF16)
                nc.vector.memset(ones1, 1.0)
            eps_emb = sing.tile([128, 1], F32)
            nc.vector.memset(eps_emb, EPS_EMB)
            eps_ln = sing.tile([128, 1], F32)
            nc.vector.memset(eps_ln, EPS_LN)

            def bcast_row(dram_row, n, dt=F32, pool=None):
                # [1, n] DRAM row -> [128, n] SBUF, replicated on partitions
                t = (pool or sing).tile([128, n], dt)
                src = bass.AP(tensor=dram_row.tensor,
                              offset=dram_row.offset, ap=[[0, 128], [1, n]])
                nc.gpsimd.dma_start(out=t, in_=src)
                return t

            type_bc = bcast_row(type0_d[0], D)
            if use_lnw:
                eg_bc = bcast_row(eg_d[0], D)
                eb_bc = bcast_row(eb_d[0], D)
            if use_mask:
                mb_bc = bcast_row(mb_d[0], S)
                mb_sb = misc.tile([128, NT], F32)
                nc.sync.dma_start(out=mb_sb,
                                  in_=mb_d[0].rearrange("(t p) -> p t", p=128))

            h_tok = hpool.tile([128, NT, D], F32)
            hT = htp.tile([128, ND, S], BF16, tag="hT")

            # ---------- layer norm helper (token-major, batched sqrt) ----------
            def layer_norm_tiles(get_x, eps_tile, g_bc, b_bc, dst_hT):
                """In-place LN on get_x(t) [128,768] f32; also writes the
                feature-major bf16 transpose into dst_hT [128,6,1024]."""
                mv_all = lnp.tile([128, NT, 2], F32, tag="mv")
                stats = lnp.tile([128, 3, 6], F32, tag="st")
                for t in range(NT):
                    xg = get_x(t).rearrange("p (n f) -> p n f", f=256)
                    for i in range(3):
                        nc.vector.bn_stats(out=stats[:, i, :], in_=xg[:, i, :])
                    nc.vector.bn_aggr(out=mv_all[:, t, :], in_=stats)
                lnv = lnp.tile([128, NT], F32, tag="lnv")
                nc.scalar.activation(out=lnv, in_=mv_all[:, :, 1], func=Ln,
                                     bias=eps_tile)
                rstd = lnp.tile([128, NT], F32, tag="rstd")
                nc.scalar.activation(out=rstd, in_=lnv, func=Exp, scale=-0.5)
                for t in range(NT):
                    o = get_x(t)
                    nc.vector.tensor_scalar(out=o, in0=o,
                                            scalar1=mv_all[:, t, 0:1],
                                            scalar2=rstd[:, t:t + 1],
                                            op0=SUB, op1=MUL)
                    if g_bc is not None:
                        nc.vector.tensor_tensor(out=o, in0=o, in1=g_bc, op=MUL)
                        nc.vector.tensor_tensor(out=o, in0=o, in1=b_bc, op=ADD)
                    bt = lnp.tile([128, D], BF16, tag="hbf")
                    nc.vector.tensor_copy(out=bt, in_=o)
                    tp = ps2.tile([128, 2048], BF16, tag="ps2")
                    for j in range(ND):
                        nc.tensor.transpose(out=tp[:, j * 128:(j + 1) * 128],
                                            in_=bt[:, j * 128:(j + 1) * 128],
                                            identity=ident)
                    nc.vector.tensor_copy(
                        out=dst_hT[:, :, t * 128:(t + 1) * 128],
                        in_=tp[:, 0:768].rearrange("p (j n) -> p j n", n=128))

            # ---------- embeddings ----------
            ids_sb = sing.tile([128, NT], I32)
            nc.sync.dma_start(out=ids_sb, in_=ids_d.rearrange("(t p) -> p t", p=128))
            for t in range(NT):
                g_t = embp.tile([128, D], F32, tag="gath")
                nc.gpsimd.indirect_dma_start(
                    out=g_t[:], out_offset=None, in_=wemb_d[:],
                    in_offset=bass.IndirectOffsetOnAxis(ap=ids_sb[:, t:t + 1], axis=0))
                p_t = embp.tile([128, D], F32, tag="pos")
                nc.sync.dma_start(out=p_t, in_=pos_d[t * 128:(t + 1) * 128, :])
                x_t = h_tok[:, t, :]
                nc.vector.tensor_tensor(out=x_t, in0=g_t, in1=p_t, op=ADD)
                nc.vector.tensor_tensor(out=x_t, in0=x_t, in1=type_bc, op=ADD)
            layer_norm_tiles(lambda t: h_tok[:, t, :], eps_emb,
                             eg_bc if use_lnw else None,
                             eb_bc if use_lnw else None, hT)

            # ---------- layers ----------
            for l in range(L):
                wq_sb = wpool.tile([128, ND, D], BF16, tag="w")
                nc.sync.dma_start(out=wq_sb,
                                  in_=wq_d[l].rearrange("(k p) n -> p k n", p=128))
                wk_sb = wpool.tile([128, ND, D], BF16, tag="w")
                nc.sync.dma_start(out=wk_sb,
                                  in_=wk_d[l].rearrange("(k p) n -> p k n", p=128))
                wv_sb = wpool.tile([128, ND, D], BF16, tag="w")
                nc.sync.dma_start(out=wv_sb,
                                  in_=wv_d[l].rearrange("(k p) n -> p k n", p=128))
                if use_bias:
                    bq_sb = bpool.tile([128, ND], F32, tag="bq")
                    nc.sync.dma_start(out=bq_sb,
                                      in_=bq_d[l].rearrange("(k p) -> p k", p=128))
                    bk_sb = bpool.tile([128, ND], F32, tag="bk")
                    nc.sync.dma_start(out=bk_sb,
                                      in_=bk_d[l].rearrange("(k p) -> p k", p=128))
                    bv_row = bpool.tile([1, D], BF16, tag="bv")
                    nc.sync.dma_start(out=bv_row, in_=bv_d[l:l + 1, :])
                    bo_row = bpool.tile([1, D], BF16, tag="bo")
                    nc.sync.dma_start(out=bo_row, in_=bo_d[l:l + 1, :])
                    b1_sb = bpool.tile([128, NC], F32, tag="b1")
                    nc.sync.dma_start(out=b1_sb,
                                      in_=b1_d[l].rearrange("(c p) -> p c", p=128))
                    b2_row = bpool.tile([1, D], BF16, tag="b2")
                    nc.sync.dma_start(out=b2_row, in_=b2_d[l:l + 1, :])
                if use_lnw:
                    l1g_bc = bcast_row(l1g_d[l, 0], D, pool=bpool)
                    l1b_bc = bcast_row(l1b_d[l, 0], D, pool=bpool)
                    l2g_bc = bcast_row(l2g_d[l, 0], D, pool=bpool)
                    l2b_bc = bcast_row(l2b_d[l, 0], D, pool=bpool)

                # --- QKV projections ---
                QT = qkvp.tile([128, ND, S], BF16, tag="QT")
                KT = qkvp.tile([128, ND, S], BF16, tag="KT")
                V_sb = qkvp.tile([128, NT, D], BF16, tag="V")
                for (W, bias_sb, out_t) in ((wq_sb, None, QT), (wk_sb, None, KT)):
                    bsb = None
                    if use_bias:
                        bsb = bq_sb if out_t is QT else bk_sb
                    for j in range(ND):
                        ps = ps2.tile([128, 1024], F32, tag="ps2")
                        for half in range(2):
                            for k in range(ND):
                                nc.tensor.matmul(
                                    out=ps[:, half * 512:(half + 1) * 512],
                                    lhsT=W[:, k, j * 128:(j + 1) * 128],
                                    rhs=hT[:, k, half * 512:(half + 1) * 512],
                                    start=(k == 0), stop=(k == ND - 1))
                        if bsb is not None:
                            nc.vector.tensor_scalar(out=out_t[:, j, :], in0=ps,
                                                    scalar1=bsb[:, j:j + 1],
                                                    scalar2=None, op0=ADD)
                        else:
                            nc.vector.tensor_copy(out=out_t[:, j, :], in_=ps)
                for t in range(NT):
                    ps = ps2.tile([128, 1024], F32, tag="ps2")
                    for nh in range(2):
                        o0 = nh * 512
                        for k in range(ND):
                            nc.tensor.matmul(
                                out=ps[:, o0:o0 + 384],
                                lhsT=hT[:, k, t * 128:(t + 1) * 128],
                                rhs=wv_sb[:, k, nh * 384:(nh + 1) * 384],
                                start=(k == 0),
                                stop=(k == ND - 1 and not use_bias))
                        if use_bias:
                            nc.tensor.matmul(
                                out=ps[:, o0:o0 + 384], lhsT=ones1,
                                rhs=bv_row[0:1, nh * 384:(nh + 1) * 384],
                                start=False, stop=True)
                    nc.vector.tensor_copy(out=V_sb[:, t, 0:384], in_=ps[:, 0:384])
                    nc.vector.tensor_copy(out=V_sb[:, t, 384:768], in_=ps[:, 512:896])

                wo_sb = wpool.tile([128, ND, D], BF16, tag="w")
                nc.sync.dma_start(out=wo_sb,
                                  in_=wo_d[l].rearrange("(k p) n -> p k n", p=128))

                # --- attention phase A: scores [q,k], exp, attn out ---
                zinv = misc.tile([128, H * NT], F32, tag="zinv")
                for hp in range(H // 2):
                    A, Bh = 2 * hp, 2 * hp + 1
                    for t in range(NT):
                        for (hd, prow, tp_) in ((A, 0, (0, 0)), (Bh, 64, (64, 0))):
                            sps = ps2.tile([128, 1024], F32, tag="ps2")
                            for half in range(2):
                                nc.tensor.matmul(
                                    out=sps[:, half * 512:(half + 1) * 512],
                                    lhsT=QT[prow:prow + 64, hp, t * 128:(t + 1) * 128],
                                    rhs=KT[prow:prow + 64, hp, half * 512:(half + 1) * 512],
                                    start=True, stop=True, tile_position=tp_)
                            if use_mask:
                                nc.vector.tensor_tensor(out=sps, in0=sps,
                                                        in1=mb_bc, op=ADD)
                            idx = hd * NT + t
                            ex = expp.tile([128, 1024], F32, tag="exp")
                            nc.scalar.activation(out=ex, in_=sps, func=Exp,
                                                 scale=0.125,
                                                 accum_out=zinv[:, idx:idx + 1])
                            nc.vector.reciprocal(out=zinv[:, idx:idx + 1],
                                                 in_=zinv[:, idx:idx + 1])
                            at = attp.tile([128, 1024], BF16, tag="attn")
                            nc.vector.tensor_scalar_mul(out=at, in0=ex,
                                                        scalar1=zinv[:, idx:idx + 1])
                            nc.scalar.dma_start(
                                out=attn_d[l, hd, t * 128:(t + 1) * 128, :], in_=at)
                # 1/Z -> DRAM transposed: zd[c, r] = zinv[r, c]
                zd = drp.tile([H * NT, 128], F32, tag="zd")
                nc.sync.dma_start(
                    out=bass.AP(tensor=zd.tensor, offset=zd.offset,
                                ap=[[1, 128], [128, H * NT]]),
                    in_=zinv)

                # --- attention phase B/C: scores [k,q], exp, ctx ---
                ctxT = qkvp.tile([128, ND, S], BF16, tag="ctxT")
                for hp in range(H // 2):
                    A, Bh = 2 * hp, 2 * hp + 1
                    izb = izp.tile([128, 2, 1024], BF16, tag="iz")
                    nc.gpsimd.dma_start(
                        out=izb.rearrange("p a q -> p (a q)"),
                        in_=bass.AP(tensor=zd.tensor,
                                    offset=zd.offset + A * NT * 128,
                                    ap=[[0, 128], [1, 2048]]))
                    cps = ps2.tile([128, 1024], F32, tag="ps2")
                    for kt in range(NT):
                        for (hd, prow, tp_, crow) in (
                                (A, 0, (0, 0), 0), (Bh, 64, (64, 0), 64)):
                            sps = ps2.tile([128, 1024], F32, tag="ps2")
                            for half in range(2):
                                nc.tensor.matmul(
                                    out=sps[:, half * 512:(half + 1) * 512],
                                    lhsT=KT[prow:prow + 64, hp, kt * 128:(kt + 1) * 128],
                                    rhs=QT[prow:prow + 64, hp, half * 512:(half + 1) * 512],
                                    start=True, stop=True, tile_position=tp_)
                            et = expt.tile([128, 1024], BF16, tag="expT")
                            if use_mask:
                                nc.scalar.activation(out=et, in_=sps, func=Exp,
                                                     scale=0.125,
                                                     bias=mb_sb[:, kt:kt + 1])
                            else:
                                nc.scalar.activation(out=et, in_=sps, func=Exp,
                                                     scale=0.125)
                            for half in range(2):
                                nc.tensor.matmul(
                                    out=cps[crow:crow + 64,
                                            half * 512:(half + 1) * 512],
                                    lhsT=V_sb[:, kt, hd * 64:(hd + 1) * 64],
                                    rhs=et[:, half * 512:(half + 1) * 512],
                                    start=(kt == 0), stop=(kt == NT - 1),
                                    tile_position=(0, crow))
                    nc.vector.tensor_tensor(out=ctxT[0:64, hp, :], in0=cps[0:64, :],
                                            in1=izb[0:64, 0, :], op=MUL)
                    nc.vector.tensor_tensor(out=ctxT[64:128, hp, :],
                                            in0=cps[64:128, :],
                                            in1=izb[64:128, 1, :], op=MUL)

                # --- Wo + residual ---
                for t in range(NT):
                    ps = ps2.tile([128, 1024], F32, tag="ps2")
                    for nh in range(2):
                        o0 = nh * 512
                        for k in range(ND):
                            nc.tensor.matmul(
                                out=ps[:, o0:o0 + 384],
                                lhsT=ctxT[:, k, t * 128:(t + 1) * 128],
                                rhs=wo_sb[:, k, nh * 384:(nh + 1) * 384],
                                start=(k == 0),
                                stop=(k == ND - 1 and not use_bias))
                        if use_bias:
                            nc.tensor.matmul(
                                out=ps[:, o0:o0 + 384], lhsT=ones1,
                                rhs=bo_row[0:1, nh * 384:(nh + 1) * 384],
                                start=False, stop=True)
                        nc.vector.tensor_tensor(
                            out=h_tok[:, t, nh * 384:(nh + 1) * 384],
                            in0=ps[:, o0:o0 + 384],
                            in1=h_tok[:, t, nh * 384:(nh + 1) * 384], op=ADD)
                h1T = htp.tile([128, ND, S], BF16, tag="hT")
                layer_norm_tiles(lambda t: h_tok[:, t, :], eps_ln,
                                 l1g_bc if use_lnw else None,
                                 l1b_bc if use_lnw else None, h1T)

                # --- FFN (token quarters of 256) ---
                for q4 in range(4):
                    o2 = [ps2.tile([128, 1024], F32, tag="ps2", name=f"o2_{q4}_{i}")
                          for i in range(2)]
                    for c in range(NC):
                        w1c = wfp.tile([128, ND, 128], BF16, tag="w1c")
                        nc.sync.dma_start(
                            out=w1c,
                            in_=w1_d[l][:, c * 128:(c + 1) * 128]
                            .rearrange("(k p) n -> p k n", p=128))
                        w2c = wfp.tile([128, D], BF16, tag="w2c")
                        nc.sync.dma_start(out=w2c, in_=w2_d[l][c * 128:(c + 1) * 128, :])
                        ups = ps2.tile([128, 1024], F32, tag="ps2")
                        for k in range(ND):
                            nc.tensor.matmul(
                                out=ups[:, 0:256],
                                lhsT=w1c[:, k, :],
                                rhs=h1T[:, k, q4 * 256:(q4 + 1) * 256],
                                start=(k == 0), stop=(k == ND - 1))
                        ub = ubfp.tile([128, 256], BF16, tag="ubf")
                        if use_bias:
                            nc.scalar.activation(out=ub, in_=ups[:, 0:256],
                                                 func=Gelu,
                                                 bias=b1_sb[:, c:c + 1])
                        else:
                            nc.scalar.activation(out=ub, in_=ups[:, 0:256],
                                                 func=Gelu)
                        for ts_ in range(2):
                            for nh in range(2):
                                nc.tensor.matmul(
                                    out=o2[ts_][:, nh * 512:nh * 512 + 384],
                                    lhsT=ub[:, ts_ * 128:(ts_ + 1) * 128],
                                    rhs=w2c[:, nh * 384:(nh + 1) * 384],
                                    start=(c == 0),
                                    stop=(c == NC - 1 and not use_bias))
                    for ts_ in range(2):
                        t = q4 * 2 + ts_
                        for nh in range(2):
                            if use_bias:
                                nc.tensor.matmul(
                                    out=o2[ts_][:, nh * 512:nh * 512 + 384],
                                    lhsT=ones1,
                                    rhs=b2_row[0:1, nh * 384:(nh + 1) * 384],
                                    start=False, stop=True)
                            nc.vector.tensor_tensor(
                                out=h_tok[:, t, nh * 384:(nh + 1) * 384],
                                in0=o2[ts_][:, nh * 512:nh * 512 + 384],
                                in1=h_tok[:, t, nh * 384:(nh + 1) * 384], op=ADD)
                hT = htp.tile([128, ND, S], BF16, tag="hT")
                layer_norm_tiles(lambda t: h_tok[:, t, :], eps_ln,
                                 l2g_bc if use_lnw else None,
                                 l2b_bc if use_lnw else None, hT)

            # ---------- pooler ----------
            wp_sb = wpool.tile([128, ND, D], BF16, tag="w")
            nc.sync.dma_start(out=wp_sb,
                              in_=wp_d.rearrange("(k p) n -> p k n", p=128))
            bp_sb = misc.tile([128, ND], F32, tag="bp")
            nc.sync.dma_start(out=bp_sb, in_=bp_d.rearrange("(j p) -> p j", p=128))
            pool_sb = misc.tile([128, ND], F32, tag="pool")
            pps = ps2.tile([128, 1024], F32, tag="ps2")
            for j in range(ND):
                for k in range(ND):
                    nc.tensor.matmul(out=pps[:, j:j + 1],
                                     lhsT=wp_sb[:, k, j * 128:(j + 1) * 128],
                                     rhs=hT[:, k, 0:1],
                                     start=(k == 0), stop=(k == ND - 1))
                nc.scalar.activation(out=pool_sb[:, j:j + 1], in_=pps[:, j:j + 1],
                                     func=Tanh, bias=bp_sb[:, j:j + 1])
            nc.sync.dma_start(out=pool_d.rearrange("(j p) -> p j", p=128),
                              in_=pool_sb)

    nc.compile()
    return nc


_CACHE = {}


def _get_kernel(flags):
    if flags not in _CACHE:
        _CACHE[flags] = build(flags)
    return _CACHE[flags]


def kernel(input_ids, attention_mask, word_emb, pos_emb, type_emb,
           emb_ln_g, emb_ln_b, Wq, bq, Wk, bk, Wv, bv, Wo, bo,
           W1, b1, W2, b2, ln1_g, ln1_b, ln2_g, ln2_b, Wp, bp,
           _trace=False, _tmpdir=None):
    input_ids = np.asarray(input_ids)
    attention_mask = np.asarray(attention_mask)
    f32 = lambda x: np.ascontiguousarray(np.asarray(x), dtype=np.float32)
    bf = lambda x: np.ascontiguousarray(np.asarray(x)).astype(NBF)

    use_mask = not bool((attention_mask == 1).all())
    use_bias = any(bool(np.asarray(b).any()) for b in (bq, bk, bv, bo, b1, b2))
    use_lnw = not (bool((np.asarray(emb_ln_g) == 1).all())
                   and not np.asarray(emb_ln_b).any()
                   and bool((np.asarray(ln1_g) == 1).all())
                   and not np.asarray(ln1_b).any()
                   and bool((np.asarray(ln2_g) == 1).all())
                   and not np.asarray(ln2_b).any())
    flags = (use_mask, use_bias, use_lnw)
    nc = _get_kernel(flags)

    shared = {
        "wemb": f32(word_emb), "pos": f32(pos_emb)[:S],
        "type0": f32(type_emb)[0:1],
        "eg": f32(emb_ln_g).reshape(1, D), "eb": f32(emb_ln_b).reshape(1, D),
        "wq": bf(Wq), "wk": bf(Wk), "wv": bf(Wv), "wo": bf(Wo),
        "w1": bf(W1), "w2": bf(W2), "wp": bf(Wp), "bp": f32(bp),
    }
    if use_bias:
        shared.update({"bq": f32(bq), "bk": f32(bk), "bv": bf(bv),
                       "bo": bf(bo), "b1": f32(b1), "b2": bf(b2)})
    if use_lnw:
        shared.update({"l1g": f32(ln1_g).reshape(L, 1, D),
                       "l1b": f32(ln1_b).reshape(L, 1, D),
                       "l2g": f32(ln2_g).reshape(L, 1, D),
                       "l2b": f32(ln2_b).reshape(L, 1, D)})

    in_maps = []
    for c in range(N_CORES):
        m = dict(shared)
        m["ids"] = np.ascontiguousarray(input_ids[c], dtype=np.int32)
        if use_mask:
            m["mb"] = ((1.0 - attention_mask[c].astype(np.float32))
                       * -1e9).reshape(1, S)
        in_maps.append(m)

    kw = {}
    if _trace:
        kw = {"trace": True, "tmpdir": _tmpdir}
    res = run_bass_kernel_spmd(nc, in_maps, core_ids=list(range(N_CORES)), **kw)

    pooled = np.stack([res.results[c]["pool"] for c in range(N_CORES)])
    attn = np.stack([res.results[c]["attn"].astype(np.float32)
                     for c in range(N_CORES)], axis=1)
    kernel.last_exec_time_ns = res.exec_time_ns
    return pooled, attn
